# revision 2
# baseline (speedup 1.0000x reference)
"""Trainium2 Bass kernel for nn_AnemllQATLinearV2 (vq_codebook).

Computes y = x @ W^T + bias where
  W[o,i] = lut[indices[o,i]] * M[o,i],
  M      = (A_dir * g) @ B_dir      (rank-4 modulation),
host-precomputed once (gather + rank-4 GEMM).

Mixed-precision GEMM, graded per out-tile by row norm: the error
budget is max-normalized, so the K-fraction each 128-row tile may run
in fp8 scales as 1/row_norm^2. Per out-shard the rows are sorted by
||W[o,:]||; tile t runs KB[t] k-tiles in bf16 and the rest as
fp8e4m3 x fp8e4m3 DoubleRow matmuls (2x PE throughput, K=256 per
instruction): KB = [14, 8, 4, 0, 0, 0, 0, 0]. Both dtypes accumulate
into one PSUM tile: the bf16 operands are pre-scaled by the same
2^18 = SX*SW factor (exact exponent shifts), one dequant at the ACT.
Measured rel err 0.0173 vs the 2e-2 gate.

Sharding over 8 NeuronCores: 2-way on out_features x 4-way on tokens.
x ships once in bf16 (blocked [128, TB, KT, 512] so per-partition DMA
runs are 1-8 KB); the fp8 copy is produced on the idle DVE right
behind each landing chunk. Input DMA triggers alternate between the
SP/Pool queues -- never the Activation queue, where a trigger stalled
on a full DMA ring would head-of-line block the PSUM drains.
"""

import sys
import types

sys.path.insert(0, "/opt/trn_rl_repo")

import numpy as np
import ml_dtypes


def _install_ntff_hook():
    try:
        import antenv.axon_hooks  # noqa: F401

        return
    except ImportError:
        pass
    try:
        from trn_agent_boot.trn_boot import _ntff_profile_via_ctypes
    except ImportError:
        return
    try:
        hook = _ntff_profile_via_ctypes("/opt/axon/libaxon_pjrt.so")
    except OSError:
        hook = None
    mod = types.ModuleType("antenv.axon_hooks")
    mod._hook = hook
    mod.get_axon_ntff_profile_hook = lambda: mod._hook
    mod.set_axon_ntff_profile_hook = lambda h: setattr(mod, "_hook", h)
    sys.modules["antenv.axon_hooks"] = mod
    import antenv

    antenv.axon_hooks = mod


_install_ntff_hook()

import concourse.bass as bass  # noqa: E402
import concourse.tile as tile  # noqa: E402
from concourse import bacc, mybir, bass_utils  # noqa: E402

# Artifact upload targets an internal bucket this environment can't reach.
bass_utils.upload_artifacts = lambda tmpdir: tmpdir

F32 = mybir.dt.float32
BF16 = mybir.dt.bfloat16
FP8 = mybir.dt.float8e4
DR = mybir.MatmulPerfMode.DoubleRow

NORM_EPS = 1e-6
MAG_EPS = 1e-6
SX = 16.0            # x scale (|x|max ~5.5 -> 88 < 240 fp8 max)
SW = 2.0 ** 14       # W scale (|W|max ~9.3e-3 -> 152 < 240)
DEQ = 1.0 / (SX * SW)

B, S, IN, OUT, R, L = 4, 2048, 2048, 2048, 4, 16
NTOK = B * S            # 8192 tokens
N_CORES = 8
TO, TT = 2, 4           # out-shard x token-shard grid
NO = OUT // TO          # 1024 out features per core
NT = NTOK // TT         # 2048 tokens per core
KT = IN // 128          # 16 K tiles
TB = NT // 512          # 4 token blocks per core
OI = NO // 128          # 8 out tiles per core
KB = [14, 8, 4, 0, 0, 0, 0, 0]   # bf16 k-tiles per out-tile (rest fp8)

# exposed for test.py
LAST_EXEC_NS = None
LAST_RESULTS = None
PROFILE = False

_PROG_CACHE = {}


def _build():
    nc = bacc.Bacc("TRN2", debug=False, target_bir_lowering=False)

    # Blocked DRAM layouts: partition-major with the transferred dims
    # contiguous per partition, so each transfer is one 0.5-8 KB run
    # per partition instead of many small descriptors.
    xb4 = nc.dram_tensor("xbB", (128, TB, KT, 512), BF16,
                         kind="ExternalInput").ap()
    wb4 = nc.dram_tensor("wbB", (128, 3, KT, 128), BF16,
                         kind="ExternalInput").ap()
    w84 = nc.dram_tensor("w8B", (128, OI, KT, 128), FP8,
                         kind="ExternalInput").ap()
    bias_d = nc.dram_tensor("biasc", (128, OI), F32, kind="ExternalInput").ap()
    yT_d = nc.dram_tensor("yT", (NO, NT), BF16, kind="ExternalOutput").ap()

    with tile.TileContext(nc) as tc:
        with (
            tc.tile_pool(name="small", bufs=1) as small,
            tc.tile_pool(name="wb", bufs=1) as wbp,
            tc.tile_pool(name="w8", bufs=1) as w8p,
            tc.tile_pool(name="xb", bufs=1) as xbp,
            tc.tile_pool(name="x8", bufs=1) as x8p,
            tc.tile_pool(name="yout", bufs=12) as yp,
            tc.tile_pool(name="yps", bufs=8, space="PSUM") as ps,
        ):
            bias_sb = small.tile([128, OI], F32)
            junk = small.tile([128, 128], BF16)
            wbt = wbp.tile([128, 3, KT, 128], BF16)
            w8t = w8p.tile([128, OI, KT, 128], FP8)
            xbt = xbp.tile([128, TB, KT, 512], BF16)
            x8t = x8p.tile([128, TB, KT, 512], FP8)

            # PE warm-up first in program order: junk matmuls (no DMA
            # dependency) ramp the PE p-state while the first W/x
            # slices land.
            nc.vector.memset(junk[:], 0.0)
            warm_ps = ps.tile([128, 512], F32, tag="py", name="warm_ps")
            for _ in range(30):
                nc.tensor.matmul(
                    warm_ps[:, 0:128], junk[:], junk[:], start=True, stop=True
                )

            eng3 = [nc.sync, nc.gpsimd, nc.scalar]
            eng2 = [nc.sync, nc.gpsimd]
            eng_i = [0]

            def dma(dst, src, three=False):
                es = eng3 if three else eng2
                es[eng_i[0] % len(es)].dma_start(dst, src)
                eng_i[0] += 1

            def cvt(tb, ksl):
                # x is pre-scaled by SX on the host, so the fp8 copy is
                # a pure dtype cast.
                nc.vector.tensor_scalar_mul(
                    x8t[:, tb, ksl, :], xbt[:, tb, ksl, :], 1.0
                )

            def xb_in(tb, ksl, three=False):
                dma(xbt[:, tb, ksl, :], xb4[:, tb, ksl, :], three)
                cvt(tb, ksl)

            # ---- input ring, in PE need-order ----
            # chase group 0 (k0-7): bf16 slices first, then the w8
            # halves its DR sub-group needs.
            dma(wbt[:, 0, 0:4, :], wb4[:, 0, 0:4, :], three=True)
            xb_in(0, slice(0, 1), three=True)
            dma(wbt[:, 1, 0:4, :], wb4[:, 1, 0:4, :], three=True)
            xb_in(0, slice(1, 2), three=True)
            dma(wbt[:, 2, 0:4, :], wb4[:, 2, 0:4, :], three=True)
            dma(bias_sb[:], bias_d[:], three=True)
            xb_in(0, slice(2, 3), three=True)
            xb_in(0, slice(3, 4), three=True)
            dma(wbt[:, 0, 4:8, :], wb4[:, 0, 4:8, :])
            xb_in(0, slice(4, 5))
            dma(wbt[:, 1, 4:8, :], wb4[:, 1, 4:8, :])
            xb_in(0, slice(5, 6))
            xb_in(0, slice(6, 7))
            xb_in(0, slice(7, 8))
            dma(w8t[:, 2, 4:8, :], w84[:, 2, 4:8, :])
            for t in range(3, OI):
                dma(w8t[:, t, 0:8, :], w84[:, t, 0:8, :])
            # chase group 1 (k8-15)
            dma(wbt[:, 0, 8:12, :], wb4[:, 0, 8:12, :])
            xb_in(0, slice(8, 10))
            dma(wbt[:, 0, 12:14, :], wb4[:, 0, 12:14, :])
            xb_in(0, slice(10, 12))
            xb_in(0, slice(12, 14))
            xb_in(0, slice(14, 16))
            dma(w8t[:, 0, 14:16, :], w84[:, 0, 14:16, :])
            dma(w8t[:, 1, 8:16, :], w84[:, 1, 8:16, :])
            dma(w8t[:, 2, 8:16, :], w84[:, 2, 8:16, :])
            for t in range(3, OI):
                dma(w8t[:, t, 8:16, :], w84[:, t, 8:16, :])
            # tb1 x in quarters so each fp8 conversion trails its
            # chunk closely; tb2/3 in halves.
            for q in range(4):
                xb_in(1, slice(q * 4, (q + 1) * 4))
            for tb in range(2, TB):
                for h in range(2):
                    xb_in(tb, slice(h * 8, (h + 1) * 8))

            # Paired drains: two out-tiles share one SBUF buffer and
            # one 512 KB y DMA. Everything sits at scale SX*SW in
            # PSUM; one dequant in the ACT.
            yT_v = yT_d.rearrange("(j p) t -> p j t", p=128)
            pend = {}

            def drain(tb, oi, py, paired=True, defer=None):
                if not paired:
                    yt = yp.tile([128, 512], BF16, tag="yt1",
                                 name=f"yt1_{tb}_{oi}")
                    nc.scalar.activation(
                        yt[:], py[:], mybir.ActivationFunctionType.Identity,
                        bias=bias_sb[:, oi:oi + 1], scale=DEQ,
                    )
                    nc.scalar.dma_start(
                        yT_d[oi * 128:(oi + 1) * 128,
                             tb * 512:(tb + 1) * 512],
                        yt[:],
                    )
                    return
                j = oi % 2
                if j == 0:
                    yt2 = yp.tile([128, 2, 512], BF16, tag="yt",
                                  name=f"yt_{tb}_{oi}")
                    pend[0] = yt2
                yt2 = pend[0]
                nc.scalar.activation(
                    yt2[:, j, :], py[:], mybir.ActivationFunctionType.Identity,
                    bias=bias_sb[:, oi:oi + 1], scale=DEQ,
                )
                if j == 1:
                    dst = yT_v[:, oi - 1:oi + 1, tb * 512:(tb + 1) * 512]
                    if defer is not None:
                        defer.append((dst, yt2))
                    else:
                        nc.scalar.dma_start(dst, yt2[:])

            def mm_b(py, t, k, tb, start):
                nc.tensor.matmul(
                    py[:], wbt[:, t, k, :], xbt[:, tb, k, :],
                    start=start, stop=False,
                )

            def mm_f(py, t, k, tb, start, stop, half=None):
                rhs = (x8t[:, tb, k:k + 2, :] if half is None else
                       x8t[:, tb, k:k + 2, half * 256:(half + 1) * 256])
                out = py[:] if half is None else py[:, 0:256]
                nc.tensor.matmul(
                    out, w8t[:, t, k:k + 2, :], rhs,
                    start=start, stop=stop, perf_mode=DR,
                )

            # tb0: k-outer chase over 8 concurrent accumulators, in two
            # 8-k groups, each group bf16-then-fp8 (few PE dtype mode
            # switches, and each k-slice is consumed as it lands).
            tb0_dmas = []
            pys0 = [
                ps.tile([128, 512], F32, tag="py", name=f"py0_{t}")
                for t in range(OI)
            ]
            for kg in (0, 8):
                for k in range(kg, kg + 8):
                    for t in range(3):
                        if k < KB[t]:
                            mm_b(pys0[t], t, k, 0, start=(k == 0))
                for k in range(kg, kg + 8, 2):
                    for t in range(OI):
                        if k >= KB[t]:
                            mm_f(pys0[t], t, k, 0,
                                 start=(k == KB[t] and t >= 3),
                                 stop=(k == KT - 2))
            # tb0 y DMAs deferred: the writes would compete with the x
            # input stream the PE is chasing.
            for t in range(OI):
                drain(0, t, pys0[t], defer=tb0_dmas)

            # tb1..3: bf16 parts of tiles 0-2 first (one mode switch),
            # then per-tile fp8 with staggered stops/drains.
            for tb in range(1, TB):
                pys = {}
                for t in range(3):
                    pys[t] = ps.tile([128, 512], F32, tag="py",
                                     name=f"py_{tb}_{t}")
                    for k in range(KB[t]):
                        mm_b(pys[t], t, k, tb, start=(k == 0))
                for t in range(OI):
                    if tb == TB - 1 and t == OI - 1:
                        continue
                    if t >= 3:
                        pys[t] = ps.tile([128, 512], F32, tag="py",
                                         name=f"py_{tb}_{t}")
                    for k in range(KB[t], KT, 2):
                        mm_f(pys[t], t, k, tb,
                             start=(k == KB[t] and t >= 3),
                             stop=(k == KT - 2))
                    defer = tb0_dmas if tb == 1 else None
                    drain(tb, t, pys[t],
                          paired=not (tb == TB - 1 and t == OI - 2),
                          defer=defer)
                    if tb == 2 and t == 0:
                        # Flush tb0+tb1 y writes: x input has landed.
                        for dst, yt2 in tb0_dmas:
                            nc.scalar.dma_start(dst, yt2[:])
                        tb0_dmas = []

            # Final tile (tb3, t7 = pure fp8): two 256-token halves in
            # separate PSUM tiles so the serial tail after the last
            # matmul is one 256-col ACT + one 64 KB DMA.
            t = OI - 1
            for h in range(2):
                ph = ps.tile([128, 512], F32, tag="py", name=f"py_last{h}")
                for k in range(0, KT, 2):
                    mm_f(ph, t, k, 3, start=(k == 0), stop=(k == KT - 2),
                         half=h)
                yt = yp.tile([128, 256], BF16, tag="yth", name=f"yt_l{h}")
                nc.scalar.activation(
                    yt[:], ph[:, 0:256],
                    mybir.ActivationFunctionType.Identity,
                    bias=bias_sb[:, t:t + 1], scale=DEQ,
                )
                nc.scalar.dma_start(
                    yT_d[t * 128:(t + 1) * 128,
                         1536 + h * 256:1792 + h * 256],
                    yt[:],
                )

    nc.compile()
    return nc


def kernel(x, indices, lut, scale_A, scale_B, rank_magnitude, bias):
    global LAST_EXEC_NS, LAST_RESULTS

    x = np.asarray(x)
    indices = np.asarray(indices)
    lut32 = np.asarray(lut, dtype=np.float32)
    scale_A = np.asarray(scale_A, dtype=np.float64)
    scale_B = np.asarray(scale_B, dtype=np.float64)
    rank_magnitude = np.asarray(rank_magnitude, dtype=np.float64)
    bias = np.asarray(bias, dtype=np.float32)

    # ---- host: W = lut[indices] * ((A_dir*g) @ B_dir) ----
    A = np.abs(scale_A)                                   # [OUT, R]
    A_dir = A / np.maximum(np.linalg.norm(A, axis=0, keepdims=True), NORM_EPS)
    Bm = np.abs(scale_B)                                  # [R, IN]
    B_dir = Bm / np.maximum(np.linalg.norm(Bm, axis=1, keepdims=True), NORM_EPS)
    g = np.log1p(np.exp(rank_magnitude)) + MAG_EPS        # softplus, [R]
    M = ((A_dir * g[None, :]) @ B_dir).astype(np.float32)  # [OUT, IN]
    W = lut32[indices] * M                                 # [OUT, IN] fp32

    # per-shard row sort by ||W[o,:]||: K-graded precision per tile
    rn = np.linalg.norm(W, axis=1)
    perms = []
    for oc in range(TO):
        rows = np.arange(oc * NO, (oc + 1) * NO)
        perms.append(rows[np.argsort(-rn[rows], kind="stable")])

    if "prog" not in _PROG_CACHE:
        _PROG_CACHE["prog"] = _build()
    nc = _PROG_CACHE["prog"]

    xT = x.reshape(NTOK, IN).T * SX                        # [IN, NTOK] fp32
    xbT = xT.astype(ml_dtypes.bfloat16)
    # blocked: token-shard -> [128, TB, KT, 512]
    xbB = xbT.reshape(KT, 128, TT, TB, 512).transpose(2, 1, 3, 0, 4)

    in_maps = []
    wcache = {}
    for c in range(N_CORES):
        oc, tc_ = c // TT, c % TT
        if oc not in wcache:
            p = perms[oc]
            Ws = W[p].T * SW                               # [IN, NO] scaled
            # bf16 big-tile blocks [128, 3, KT, 128]
            wbB = np.zeros((128, 3, KT, 128), dtype=ml_dtypes.bfloat16)
            w8B = np.zeros((128, OI, KT, 128), dtype=ml_dtypes.float8_e4m3)
            for t in range(OI):
                Wt = Ws[:, t * 128:(t + 1) * 128]          # [IN, 128]
                Wt3 = Wt.reshape(KT, 128, 128).transpose(1, 0, 2)
                if t < 3 and KB[t] > 0:
                    wbB[:, t, :KB[t], :] = Wt3[:, :KB[t], :].astype(
                        ml_dtypes.bfloat16)
                w8B[:, t, KB[t]:, :] = np.clip(
                    Wt3[:, KB[t]:, :], -240, 240).astype(
                    ml_dtypes.float8_e4m3)
            wcache[oc] = (
                np.ascontiguousarray(wbB),
                np.ascontiguousarray(w8B),
                np.ascontiguousarray(bias[p].reshape(OI, 128).T),
            )
        wbB, w8B, biasc = wcache[oc]
        in_maps.append({
            "xbB": np.ascontiguousarray(xbB[tc_]),
            "wbB": wbB,
            "w8B": w8B,
            "biasc": biasc,
        })

    res = bass_utils.run_bass_kernel_spmd(
        nc, in_maps, core_ids=list(range(N_CORES)), trace=PROFILE
    )
    LAST_EXEC_NS = res.exec_time_ns
    LAST_RESULTS = res

    # ---- host: gather + un-permute ----
    y = np.empty((NTOK, OUT), dtype=np.float32)
    for c in range(N_CORES):
        oc, tc_ = c // TT, c % TT
        yT_c = res.results[c]["yT"]                       # [NO(sorted), NT]
        y[tc_ * NT:(tc_ + 1) * NT, perms[oc]] = yT_c.T
    return y.reshape(B, S, OUT)
